# revision 8
# baseline (speedup 1.0000x reference)
"""Two-layer GAT (graph attention network) on 8 Trainium2 NeuronCores.

Strategy (see spec sharding_hint):
  * Edges are sorted by destination node; destination nodes (and their
    incoming edges) are partitioned across the 8 cores (6250 nodes each).
  * Dense parts (z = x @ W, attention logit projections es/ed) are computed
    as node-sharded matmuls with host-fused weights:
        W_aug = [W | W@a_src(per head) | W@a_dst(per head)]
    so one matmul yields z, es, ed for each node.
  * Node tables (z|es and ed) are replicated to every core's HBM; the edge
    phase indirect-DMA-gathers per-edge rows T[src] and ED[dst], computes
    w = exp(leakyrelu(es_src + ed_dst)), and segment-sums messages
    [w*z | w] into per-destination accumulators using one-hot matmuls
    accumulated in PSUM (host assigns each destination node a slot in a
    <=128-node group; each group's edges are padded to whole 128-edge
    chunks).  The trailing w-columns give the softmax denominators; the
    epilogue divides and (layer 1) applies ELU, then scatter-writes rows.
  * Softmax max-subtraction is skipped: attention logits are O(1) here so
    exp() cannot overflow, and the result is mathematically identical.
  * The inter-layer all-gather of the (small) node tables is done on host
    between the four device launches (dense1, edge1, dense2, edge2).
"""

import os
import sys

import numpy as np

for _p in ("/opt/trn_rl_repo", "/root/.axon_site/_ro/trn_rl_repo"):
    if os.path.isdir(_p) and _p not in sys.path:
        sys.path.insert(0, _p)

# ---------------------------------------------------------------- constants
N = 50000
E = 800000
IN_DIM = 128
HID = 16
HEADS = 8
OUT_DIM = 32
NEG_SLOPE = 0.2

CORES = 8
NPC = N // CORES          # nodes per core
P = 128                   # partitions == edges per chunk
CPG = 16                  # chunks per group (<=128 dst slots, 2048 edge slots)
SUPER = 4                 # groups per indirect-gather instruction
DUMMY = N                 # table row for padding edges (z=0, es=-1e4 -> w=0)
TRASH = NPC               # output row for padding slots
ES_PAD = -1.0e4           # es value of the dummy row: exp(0.2*es) underflows to 0
DENSE_W = 144             # dense output row: z(<=128) | es(<=8) | ed(<=8)
NT = (NPC + P - 1) // P   # dense node tiles per core (49)

_EDGE_PLAN_CACHE = {}


# ---------------------------------------------------------------- host prep
def fuse_weights(W, a_src, a_dst, H, D):
    """W:[K, H*D] -> [K, H*D + 2H] = [W | Wes | Wed] padded to DENSE_W."""
    K = W.shape[0]
    Wr = W.reshape(K, H, D)
    wes = np.einsum("khd,hd->kh", Wr, a_src)
    wed = np.einsum("khd,hd->kh", Wr, a_dst)
    out = np.zeros((K, DENSE_W), dtype=np.float32)
    out[:, : H * D] = W
    out[:, H * D : H * D + H] = wes
    out[:, H * D + H : H * D + 2 * H] = wed
    return out


def build_edge_plan(src, dst):
    """Sort edges by dst, shard by dst-core, group into <=128-node /
    <=CPG*P-edge groups, and emit per-core uniform index arrays.

    Returns dict with NS (supergroups per core) and per-core arrays:
      srci  [NS, P, SUPER*CPG] int32  table row of edge's src (DUMMY pad)
      dsti  [NS, P, SUPER*CPG] int32  table row of edge's dst (DUMMY pad)
      slot  [NS, P, SUPER*CPG] f32    dst slot within group (0 pad)
      oidx  [NS, P, SUPER]     int32  output row per slot (TRASH pad)
    """
    key = (src.tobytes(), dst.tobytes())
    h = hash(key)
    if h in _EDGE_PLAN_CACHE:
        return _EDGE_PLAN_CACHE[h]

    order = np.argsort(dst, kind="stable")
    ssrc = src[order].astype(np.int64)
    sdst = dst[order].astype(np.int64)
    deg = np.bincount(dst, minlength=N).astype(np.int64)
    starts = np.zeros(N + 1, dtype=np.int64)
    np.cumsum(deg, out=starts[1:])

    core_groups = []
    for c in range(CORES):
        lo, hi = c * NPC, (c + 1) * NPC
        groups = []
        n = lo
        while n < hi:
            n0 = n
            ecnt = 0
            while n < hi and (n - n0) < P and ecnt + deg[n] <= CPG * P:
                ecnt += deg[n]
                n += 1
            assert n > n0, f"node {n} degree {deg[n]} exceeds group capacity"
            groups.append((n0, n - n0, starts[n0], ecnt))
        core_groups.append(groups)

    G = max(len(g) for g in core_groups)
    G = ((G + SUPER - 1) // SUPER) * SUPER
    NS = G // SUPER

    plan = {"NS": NS, "cores": []}
    for c in range(CORES):
        srci = np.full((NS, P, SUPER * CPG), DUMMY, dtype=np.int32)
        dsti = np.full((NS, P, SUPER * CPG), DUMMY, dtype=np.int32)
        slot = np.zeros((NS, P, SUPER * CPG), dtype=np.float32)
        oidx = np.full((NS, P, SUPER), TRASH, dtype=np.int32)
        for g, (n0, ncnt, e0, ecnt) in enumerate(core_groups[c]):
            s, gg = divmod(g, SUPER)
            j = np.arange(ecnt)
            pp = j % P
            cc = j // P
            col = gg * CPG + cc
            srci[s, pp, col] = ssrc[e0 : e0 + ecnt]
            dsti[s, pp, col] = sdst[e0 : e0 + ecnt]
            slot[s, pp, col] = (sdst[e0 : e0 + ecnt] - n0).astype(np.float32)
            oidx[s, :ncnt, gg] = (n0 - c * NPC) + np.arange(ncnt)
        plan["cores"].append(
            {"srci": srci, "dsti": dsti, "slot": slot, "oidx": oidx}
        )
    _EDGE_PLAN_CACHE[h] = plan
    return plan


# ---------------------------------------------------------------- builders
def _bass_mods():
    import concourse.bass as bass
    import concourse.tile as tile
    from concourse import mybir

    return bass, tile, mybir


_SAFE_TC = None


def _safe_tile_context():
    """TileContext whose kernel-tail drain never carries more than 2 sem
    waits per instruction (this container's walrus rejects >2 sync-wait
    commands on the SP CTRL drain); excess waits are moved onto preceding
    SP nops."""
    global _SAFE_TC
    if _SAFE_TC is not None:
        return _SAFE_TC
    import concourse.tile as tile
    from concourse import mybir
    from concourse.vector_clock import ScopedClock

    class TileContextSafe(tile.TileContext):
        def _add_instruction(self, inst):
            # This container's walrus accepts at most ONE sync-wait command
            # per instruction; hoist extra waits onto preceding same-engine
            # nops (same semantics: engine blocks on them in order).
            si = inst.sync_info
            if (
                si is not None
                and si.on_wait
                and len(si.on_wait) > 1
                and inst.engine != mybir.EngineType.Unassigned
            ):
                waits = list(si.on_wait)
                si.on_wait = waits[-1:]
                for w in waits[:-1]:
                    nop = mybir.InstNoOp(
                        name=self.nc.get_next_instruction_name(), ins=[], outs=[]
                    )
                    nop.engine = inst.engine
                    nop.sync_info = mybir.SyncInfo(on_wait=[w], on_update=[])
                    super()._add_instruction(nop)
            super()._add_instruction(inst)

        def _drain_and_barrier(self, tick_clock, wait_clock):
            nc = self.nc
            nops = [nc.sync.nop(nofuse=True) for _ in range(28)]
            drain_inst = nc.sync.drain()
            wait_clock.add_sem_waits(
                drain_inst.ins, ScopedClock({None: tick_clock.global_clock})
            )
            si = drain_inst.ins.sync_info
            waits = list(si.on_wait) if si is not None and si.on_wait else []
            if len(waits) > 1:
                si.on_wait = waits[:1]
                rest = waits[1:]
                assert len(rest) <= len(nops), "raise nop count"
                for k, w in enumerate(rest):
                    nops[k].ins.sync_info = mybir.SyncInfo(
                        on_wait=[w], on_update=[]
                    )

            nc.all_engine_barrier()
            assert self.sems is not None
            popped = nc._tile_sem_poison_stack.pop()
            assert popped is self._sem_poison
            nc.clear_and_free_semaphores(list(self.sems.allocated().values()))
            nc.all_engine_barrier()

    _SAFE_TC = TileContextSafe
    return _SAFE_TC


def _set_queue(ins, i):
    # multi-queue SWDGE measured no faster (desc-gen is serial on Q7);
    # keep everything on the default queue.
    return ins


def _ap(tile_ap, col_off, dims):
    """Custom free-dim AP on an SBUF tile: keep the tile's partition dim,
    replace free dims with [step, count] pairs (steps in elements)."""
    import concourse.bass as bass

    part = list(tile_ap.ap[0])
    return bass.AP(
        tile_ap.tensor,
        tile_ap.offset + col_off,
        [part] + [list(d) for d in dims],
    )


def build_dense_nc():
    """Per-core dense phase: out[tile] = xT[:, tile].T @ Waug  -> [NT*P, 144].

    xT: [128, NT*P] f32 (column-padded transposed features for this core's
    nodes), waug: [128, 144].
    """
    bass, tile, mybir = _bass_mods()
    f32 = mybir.dt.float32
    nc = bass.Bass("TRN2")
    xT = nc.dram_tensor("xt", [P, NT * P], f32, kind="ExternalInput")
    W = nc.dram_tensor("waug", [P, DENSE_W], f32, kind="ExternalInput")
    OUTD = nc.dram_tensor("outd", [NT * P, DENSE_W], f32, kind="ExternalOutput")

    with _safe_tile_context()(nc) as tc:
        from contextlib import ExitStack

        with ExitStack() as ctx:
            const = ctx.enter_context(tc.tile_pool(name="const", bufs=1))
            work = ctx.enter_context(tc.tile_pool(name="work", bufs=3))
            psum = ctx.enter_context(tc.tile_pool(name="psum", bufs=4, space="PSUM"))

            wsb = const.tile([P, DENSE_W], f32)
            nc.sync.dma_start(out=wsb[:], in_=W[:, :])
            xsb = const.tile([P, NT * P], f32)
            nc.sync.dma_start(out=xsb[:], in_=xT[:, :])

            for t in range(NT):
                ps = psum.tile([P, DENSE_W], f32, tag="ps")
                nc.tensor.matmul(
                    out=ps[:],
                    lhsT=xsb[:, t * P : (t + 1) * P],
                    rhs=wsb[:],
                    start=True,
                    stop=True,
                )
                st = work.tile([P, DENSE_W], f32, tag="st")
                nc.vector.tensor_copy(out=st[:], in_=ps[:])
                nc.sync.dma_start(out=OUTD[t * P : (t + 1) * P, :], in_=st[:])
    return nc


def build_edge_nc(NS, RW, H, D, elu):
    """Per-core edge phase for one GAT layer.

    T:  [N+1, RW] f32, row = [z (H*D) | es (H)]; row N is the pad row.
    ED: [N+1, H]  f32 ed table; row N is zero.
    Aggregates alpha-weighted messages per destination into OUT [NPC+1, ZW].
    """
    bass, tile, mybir = _bass_mods()
    from contextlib import ExitStack

    f32 = mybir.dt.float32
    i32 = mybir.dt.int32
    ZW = H * D
    MW = ZW + H
    COLS = SUPER * CPG

    nc = bass.Bass("TRN2")
    T = nc.dram_tensor("tbl", [N + 1, RW], f32, kind="ExternalInput")
    ED = nc.dram_tensor("edt", [N + 1, H], f32, kind="ExternalInput")
    SRC = nc.dram_tensor("srci", [NS, P, COLS], i32, kind="ExternalInput")
    DST = nc.dram_tensor("dsti", [NS, P, COLS], i32, kind="ExternalInput")
    SLOT = nc.dram_tensor("slot", [NS, P, COLS], f32, kind="ExternalInput")
    OIDX = nc.dram_tensor("oidx", [NS, P, SUPER], i32, kind="ExternalInput")
    IOTA = nc.dram_tensor("iota", [P, P], f32, kind="ExternalInput")
    OUT = nc.dram_tensor("out", [NPC + 1, ZW], f32, kind="ExternalOutput")

    with _safe_tile_context()(nc) as tc:
        with ExitStack() as ctx:
            const = ctx.enter_context(tc.tile_pool(name="const", bufs=1))
            meta = ctx.enter_context(tc.tile_pool(name="meta", bufs=2))
            gath = ctx.enter_context(tc.tile_pool(name="gath", bufs=2))
            work = ctx.enter_context(tc.tile_pool(name="work", bufs=2))
            psum = ctx.enter_context(tc.tile_pool(name="psum", bufs=4, space="PSUM"))

            iota = const.tile([P, P], f32)
            nc.sync.dma_start(out=iota[:], in_=IOTA[:, :])

            for s in range(NS):
                srci = meta.tile([P, COLS], i32, tag="srci")
                nc.sync.dma_start(out=srci[:], in_=SRC[s, :, :])
                dsti = meta.tile([P, COLS], i32, tag="dsti")
                nc.sync.dma_start(out=dsti[:], in_=DST[s, :, :])
                slot = meta.tile([P, COLS], f32, tag="slot")
                nc.sync.dma_start(out=slot[:], in_=SLOT[s, :, :])
                oidx = meta.tile([P, SUPER], i32, tag="oidx")
                nc.sync.dma_start(out=oidx[:], in_=OIDX[s, :, :])

                for gg in range(SUPER):
                    # per-chunk single-index-per-partition indirect gathers
                    # (the HW contract: 1 offset per partition, contiguous
                    # num_elem read) into group-contiguous tiles
                    gt = gath.tile([P, CPG * RW], f32, tag="gt")
                    edt_g = gath.tile([P, CPG * H], f32, tag="edt")
                    for cch in range(CPG):
                        col = gg * CPG + cch
                        ins = nc.gpsimd.indirect_dma_start(
                            out=gt[:, cch * RW : (cch + 1) * RW],
                            out_offset=None,
                            in_=T[:, :],
                            in_offset=bass.IndirectOffsetOnAxis(
                                ap=srci[:, col : col + 1], axis=0
                            ),
                        )
                        _set_queue(ins, col)
                        ins = nc.gpsimd.indirect_dma_start(
                            out=edt_g[:, cch * H : (cch + 1) * H],
                            out_offset=None,
                            in_=ED[:, :],
                            in_offset=bass.IndirectOffsetOnAxis(
                                ap=dsti[:, col : col + 1], axis=0
                            ),
                        )
                        _set_queue(ins, col)
                    gbase = 0
                    EC = CPG * H
                    # ---- attention weights: w = exp(lrelu(es_src + ed_dst))
                    e_t = work.tile([P, EC], f32, tag="e")
                    nc.vector.tensor_tensor(
                        out=e_t[:],
                        in0=_ap(gt[:], gbase + ZW, [[RW, CPG], [1, H]]),
                        in1=edt_g[:],
                        op=mybir.AluOpType.add,
                    )
                    t2 = work.tile([P, EC], f32, tag="t2")
                    nc.vector.tensor_scalar_mul(t2[:], e_t[:], NEG_SLOPE)
                    t3 = work.tile([P, EC], f32, tag="t3")
                    nc.vector.tensor_tensor(
                        out=t3[:], in0=e_t[:], in1=t2[:], op=mybir.AluOpType.max
                    )
                    w_t = work.tile([P, EC], f32, tag="w")
                    nc.scalar.activation(
                        out=w_t[:], in_=t3[:], func=mybir.ActivationFunctionType.Exp
                    )
                    # ---- one-hot chunk matrices [P, CPG*P]
                    oh = work.tile([P, CPG * P], f32, tag="oh")
                    nc.vector.tensor_tensor(
                        out=oh[:],
                        in0=_ap(slot[:], gg * CPG, [[1, CPG], [0, P]]),
                        in1=_ap(iota[:], 0, [[0, CPG], [1, P]]),
                        op=mybir.AluOpType.is_equal,
                    )
                    # ---- messages M = [w*z | w] per chunk
                    m_t = work.tile([P, CPG * MW], f32, tag="m")
                    nc.vector.tensor_tensor(
                        out=_ap(m_t[:], 0, [[MW, CPG], [1, ZW]]),
                        in0=_ap(gt[:], gbase, [[RW, CPG], [1, ZW]]),
                        in1=_ap(w_t[:], 0, [[H, CPG], [1, H], [0, D]]),
                        op=mybir.AluOpType.mult,
                    )
                    nc.vector.tensor_copy(
                        out=_ap(m_t[:], ZW, [[MW, CPG], [1, H]]),
                        in_=_ap(w_t[:], 0, [[H, CPG], [1, H]]),
                    )
                    # ---- segment-sum into PSUM via one-hot matmuls
                    ps = psum.tile([P, MW], f32, tag="ps")
                    for cch in range(CPG):
                        nc.tensor.matmul(
                            out=ps[:],
                            lhsT=oh[:, cch * P : (cch + 1) * P],
                            rhs=m_t[:, cch * MW : (cch + 1) * MW],
                            start=(cch == 0),
                            stop=(cch == CPG - 1),
                        )
                    # ---- epilogue: divide by denominator (+ ELU on layer 1)
                    sden = work.tile([P, H], f32, tag="sden")
                    nc.vector.tensor_scalar_add(sden[:], ps[:, ZW:MW], 1e-30)
                    rs = work.tile([P, H], f32, tag="rs")
                    nc.vector.reciprocal(rs[:], sden[:])
                    h1 = work.tile([P, ZW], f32, tag="h1")
                    nc.vector.tensor_tensor(
                        out=h1[:],
                        in0=ps[:, :ZW],
                        in1=_ap(rs[:], 0, [[1, H], [0, D]]),
                        op=mybir.AluOpType.mult,
                    )
                    if elu:
                        pos = work.tile([P, ZW], f32, tag="pos")
                        nc.vector.tensor_scalar_max(pos[:], h1[:], 0.0)
                        ngx = work.tile([P, ZW], f32, tag="ngx")
                        nc.vector.tensor_scalar_min(ngx[:], h1[:], 0.0)
                        ex = work.tile([P, ZW], f32, tag="ex")
                        nc.scalar.activation(
                            out=ex[:],
                            in_=ngx[:],
                            func=mybir.ActivationFunctionType.Exp,
                        )
                        hf = work.tile([P, ZW], f32, tag="hf")
                        nc.vector.tensor_tensor(
                            out=hf[:], in0=pos[:], in1=ex[:], op=mybir.AluOpType.add
                        )
                        out_t = work.tile([P, ZW], f32, tag="outt")
                        nc.vector.tensor_scalar_add(out_t[:], hf[:], -1.0)
                    else:
                        out_t = h1
                    nc.gpsimd.indirect_dma_start(
                        out=OUT[:, :],
                        out_offset=bass.IndirectOffsetOnAxis(
                            ap=oidx[:, gg : gg + 1], axis=0
                        ),
                        in_=out_t[:],
                        in_offset=None,
                    )
    return nc


# ---------------------------------------------------------------- run layer
def _run_spmd(nc, in_maps, collect, label):
    from concourse.bass_utils import run_bass_kernel_spmd

    trace = bool(int(os.environ.get("GAT_TRACE", "0")))
    res = run_bass_kernel_spmd(
        nc, in_maps, core_ids=list(range(CORES)), trace=trace
    )
    if collect is not None:
        collect.append((label, getattr(res, "exec_time_ns", None)))
    return res.results


def _dense_phase(x, Waug, plan_unused, collect, label):
    """x: [N, 128] f32. Returns full tables z|es [N,?], ed [N,?] stacked as
    the raw [N, DENSE_W] dense output."""
    xT = np.ascontiguousarray(x.T.astype(np.float32))  # [128, N]
    xT_pad = np.zeros((P, NT * P), dtype=np.float32)
    in_maps = []
    for c in range(CORES):
        xc = np.array(xT_pad)
        xc[:, :NPC] = xT[:, c * NPC : (c + 1) * NPC]
        in_maps.append({"xt": xc, "waug": Waug})
    outs = _run_spmd(build_dense_nc(), in_maps, collect, label)
    full = np.concatenate([o["outd"][:NPC] for o in outs], axis=0)
    return full  # [N, DENSE_W]


def _edge_phase(dense_full, plan, RW, H, D, elu, collect, label):
    ZW = H * D
    tbl = np.zeros((N + 1, RW), dtype=np.float32)
    tbl[:N, :ZW] = dense_full[:, :ZW]
    tbl[:N, ZW : ZW + H] = dense_full[:, ZW : ZW + H]
    tbl[N, ZW : ZW + H] = ES_PAD
    edt = np.zeros((N + 1, H), dtype=np.float32)
    edt[:N] = dense_full[:, ZW + H : ZW + 2 * H]
    iota = np.broadcast_to(
        np.arange(P, dtype=np.float32)[None, :], (P, P)
    ).copy()
    in_maps = []
    for c in range(CORES):
        pc = plan["cores"][c]
        in_maps.append(
            {
                "tbl": tbl,
                "edt": edt,
                "srci": pc["srci"],
                "dsti": pc["dsti"],
                "slot": pc["slot"],
                "oidx": pc["oidx"],
                "iota": iota,
            }
        )
    outs = _run_spmd(build_edge_nc(plan["NS"], RW, H, D, elu), in_maps, collect, label)
    return np.concatenate([o["out"][:NPC] for o in outs], axis=0)  # [N, ZW]


# ---------------------------------------------------------------- kernel
def kernel(h, W1, a1_src, a1_dst, W2, a2_src, a2_dst, src, dst, _collect=None):
    h = np.asarray(h, dtype=np.float32)
    W1 = np.asarray(W1, dtype=np.float32)
    W2 = np.asarray(W2, dtype=np.float32)
    a1_src = np.asarray(a1_src, dtype=np.float32)
    a1_dst = np.asarray(a1_dst, dtype=np.float32)
    a2_src = np.asarray(a2_src, dtype=np.float32)
    a2_dst = np.asarray(a2_dst, dtype=np.float32)
    src = np.asarray(src)
    dst = np.asarray(dst)

    plan = build_edge_plan(src, dst)
    W1a = fuse_weights(W1, a1_src, a1_dst, HEADS, HID)
    W2a = fuse_weights(W2, a2_src, a2_dst, 1, OUT_DIM)

    d1 = _dense_phase(h, W1a, plan, _collect, "dense1")
    h1 = _edge_phase(d1, plan, RW=136, H=HEADS, D=HID, elu=True,
                     collect=_collect, label="edge1")
    d2 = _dense_phase(h1, W2a, plan, _collect, "dense2")
    out = _edge_phase(d2, plan, RW=33, H=1, D=OUT_DIM, elu=False,
                      collect=_collect, label="edge2")
    return out.astype(np.float32)
